# revision 11
# baseline (speedup 1.0000x reference)
"""Trainium2 Bass kernel for nn_CentersDistance (retrieval_knn).

logits[k, n] = -||centers[k] - inputs[n]||^2
             = 2*(centers @ inputs.T)[k, n] - ||centers[k]||^2 - ||inputs[n]||^2

Strategy (8 NeuronCores, data-parallel over the N=8192 inputs):
  * device computes ONLY the cross term 2*c.x as 64 fp8e4m3 DoubleRow
    matmuls per core (contraction 256/MM via the [p, 2, f] interleave,
    2 fp8 MACs/cell/cycle -> 216 ns/MM warm = the DR stream floor), PSUM
    fp32, copied to SBUF as fp16 and stored; the exact norm terms
    (float64 on host) are added on the host after gather.
  * group order in "quarters": (m0-3,h0), (m0-3,h1), (m4-7,h0),
    (m4-7,h1); within a quarter the matmuls run j-major (all 4 groups'
    DR-step j before step j+1) so the PE starts on the first 128 KB
    chunks as soon as they land and chunk consumption (~148 GB/s
    equivalent warm) stays just under the 2-ring delivery (~270 GB/s
    aggregate measured under 8-core contention).
  * loads in 128 KB chunks: ct (j, m-half) on the Scalar ring, xt (j, h)
    on the Sync ring, one semaphore per chunk.  PSUM->SBUF fp16 copies
    alternate DVE (even retirement order) / ACT (odd); stores alternate
    Sync (even, gated on dve_cp) / ACT (odd, engine-ordered after its
    own copy, no semaphore).  No final store wait: the NRT postamble's
    DMA drain covers the last in-flight stores, so the engines enter the
    postamble ~2 us earlier.
  * N_WU throwaway matmuls on an uninitialized scratch tile keep the PE
    busy from preamble-end to first-chunk-arrival so the HAM clock gate
    (~3.4 us sustained-busy window) opens before the real matmuls start.
"""

import threading
from contextlib import ExitStack

import numpy as np
import ml_dtypes

import concourse.mybir as mybir
from concourse import bacc
from concourse.bass_utils import run_bass_kernel_spmd

N_CORES = 8
N, K, D = 8192, 1024, 1024
NSH = N // N_CORES  # per-core slab of inputs
P = 128             # SBUF partitions
NF = 512            # matmul moving free dim (one fp32 PSUM bank)

KS = D // P         # 8 contraction subtiles of 128
J = KS // 2         # 4 DoubleRow steps of 256
M_TILES = K // P    # 8 center tiles
H_TILES = NSH // NF # 2 moving-dim tiles

G = M_TILES * H_TILES  # 16 output groups of [128, 512]
N_WU = 8               # PE warm-up matmuls

# group order: quarters (m0-3,h0), (m0-3,h1), (m4-7,h0), (m4-7,h1)
GROUP_ORDER = (
    [(m, 0) for m in range(4)]
    + [(m, 1) for m in range(4)]
    + [(m, 0) for m in range(4, 8)]
    + [(m, 1) for m in range(4, 8)]
)

_DT = mybir.dt.float8e4
_NP_DT = ml_dtypes.float8_e4m3

_cache = threading.local()


def _build_nc(dt=_DT, n_wu=N_WU):
    nc = bacc.Bacc(
        "TRN2", target_bir_lowering=False, debug=False, num_devices=N_CORES
    )
    ct = nc.dram_tensor("ct", [P, KS, K], dt, kind="ExternalInput").ap()
    xt = nc.dram_tensor("xt", [P, KS, NSH], dt, kind="ExternalInput").ap()
    out = nc.dram_tensor("out", [K, NSH], mybir.dt.float16, kind="ExternalOutput").ap()

    out_r = out.rearrange("(m p) n -> m p n", p=P)

    DR = mybir.MatmulPerfMode.DoubleRow

    # 128 KB load chunks, in ring order (first-consumed first).
    # ct chunk (j, mhalf): Scalar ring, order mlo j0-3 then mhi j0-3.
    # xt chunk (j, h):     Sync ring, order h0 j0-3 then h1 j0-3.
    def ct_chunk(j, m):  # -> chunk index on the Scalar ring
        return (m // 4) * J + j

    def xt_chunk(j, h):  # -> chunk index on the Sync ring
        return h * J + j

    with (
        nc.sbuf_tensor("wu_sb", [P, NF], dt) as wu_sb,
        nc.sbuf_tensor("ct_sb", [P, KS, K], dt) as ct_sb,
        nc.sbuf_tensor("xt_sb", [P, KS, NSH], dt) as xt_sb,
        nc.sbuf_tensor("ot_sb", [P, G * NF], mybir.dt.float16) as ot_sb,
        ExitStack() as stack,
        nc.semaphore("mm_sem") as mm_sem,
        nc.semaphore("dve_cp") as dve_cp,
        nc.semaphore("act_cp") as act_cp,
        nc.semaphore("dma_out") as dma_out,
        nc.Block() as block,
    ):
        ct_sems = [
            stack.enter_context(nc.semaphore(f"ct_sem{i}")) for i in range(2 * J)
        ]
        xt_sems = [
            stack.enter_context(nc.semaphore(f"xt_sem{i}")) for i in range(2 * J)
        ]
        ps = [
            stack.enter_context(nc.psum_tensor(f"ps{b}", [P, NF], mybir.dt.float32))
            for b in range(8)
        ]

        cp_sem = {0: dve_cp, 1: act_cp}  # order parity -> copy engine sem

        @block.sync
        def _(sync):
            for h in range(2):
                for j in range(J):
                    sync.dma_start(
                        xt_sb[:, 2 * j : 2 * j + 2, h * NF : (h + 1) * NF],
                        xt[:, 2 * j : 2 * j + 2, h * NF : (h + 1) * NF],
                    ).then_inc(xt_sems[xt_chunk(j, h)], 16)
            # even-order stores, gated on the DVE copy
            for o, (m, h) in enumerate(GROUP_ORDER):
                if o % 2 != 0:
                    continue
                sync.wait_ge(dve_cp, (o // 2) + 1)
                sync.dma_start(
                    out_r[m][:, h * NF : (h + 1) * NF],
                    ot_sb[:, o * NF : (o + 1) * NF],
                ).then_inc(dma_out, 16)

        @block.scalar
        def _(scalar):
            for mhalf in range(2):
                for j in range(J):
                    kl = mhalf * 4 * P
                    scalar.dma_start(
                        ct_sb[:, 2 * j : 2 * j + 2, kl : kl + 4 * P],
                        ct[:, 2 * j : 2 * j + 2, kl : kl + 4 * P],
                    ).then_inc(ct_sems[ct_chunk(j, 4 * mhalf)], 16)
            # odd-order copies (PSUM -> SBUF fp16) on ACT, each followed
            # in engine order by its own store (no semaphore needed)
            for o, (m, h) in enumerate(GROUP_ORDER):
                if o % 2 != 1:
                    continue
                scalar.wait_ge(mm_sem, o + 1)
                nc.scalar.copy(
                    ot_sb[:, o * NF : (o + 1) * NF], ps[o % 8][:]
                ).then_inc(act_cp, 1)
                # the DMA descriptor fetch can race the ACT pipeline's SBUF
                # write, so gate on the copy's semaphore even on-engine
                scalar.wait_ge(act_cp, (o // 2) + 1)
                scalar.dma_start(
                    out_r[m][:, h * NF : (h + 1) * NF],
                    ot_sb[:, o * NF : (o + 1) * NF],
                ).then_inc(dma_out, 16)

        @block.tensor
        def _(tensor):
            # warm-up: keep the PE busy from preamble-end to first-chunk
            # arrival so the HAM clock gate opens early.  wu_sb is
            # deliberately uninitialized; bank 7 is rewritten with
            # start=True by order-7's first matmul much later.
            for _ in range(n_wu):
                nc.tensor.matmul(
                    ps[7][:], wu_sb[:, 0:P], wu_sb[:], start=True, stop=True
                )
            ct_waited = set()
            xt_waited = set()
            for q in range(4):  # quarter
                quarter = list(enumerate(GROUP_ORDER))[4 * q : 4 * q + 4]
                for j in range(J):
                    for o, (m, h) in quarter:
                        ci = ct_chunk(j, m)
                        if ci not in ct_waited:
                            ct_waited.add(ci)
                            tensor.wait_ge(ct_sems[ci], 16)
                        xi = xt_chunk(j, h)
                        if xi not in xt_waited:
                            xt_waited.add(xi)
                            tensor.wait_ge(xt_sems[xi], 16)
                        if j == 0 and o >= 8:
                            # bank o%8 was last drained by order o-8's copy
                            tensor.wait_ge(
                                cp_sem[(o - 8) % 2], ((o - 8) // 2) + 1
                            )
                        mm = nc.tensor.matmul(
                            ps[o % 8][:],
                            ct_sb[:, 2 * j : 2 * j + 2, m * P : (m + 1) * P],
                            xt_sb[:, 2 * j : 2 * j + 2, h * NF : (h + 1) * NF],
                            start=(j == 0),
                            stop=(j == J - 1),
                            perf_mode=DR,
                        )
                        if j == J - 1:
                            mm.then_inc(mm_sem, 1)

        @block.vector
        def _(vector):
            # even-order copies (PSUM -> SBUF fp16) on the DVE
            for o, (m, h) in enumerate(GROUP_ORDER):
                if o % 2 != 0:
                    continue
                vector.wait_ge(mm_sem, o + 1)
                nc.vector.tensor_copy(
                    ot_sb[:, o * NF : (o + 1) * NF], ps[o % 8][:]
                ).then_inc(dve_cp, 1)

    nc.compile()
    return nc


def _get_nc():
    if not hasattr(_cache, "nc"):
        _cache.nc = _build_nc()
    return _cache.nc


def _pack_dxf(a_t):
    """[D, F] -> [128, KS, F] with d = ks*128 + p."""
    Dd, F = a_t.shape
    return np.ascontiguousarray(a_t.reshape(KS, P, F).transpose(1, 0, 2))


def kernel(inputs, centers, _trace=False):
    inputs = np.asarray(inputs, dtype=np.float32)
    centers = np.asarray(centers, dtype=np.float32)

    csq = np.sum(centers.astype(np.float64) ** 2, axis=1)
    xsq = np.sum(inputs.astype(np.float64) ** 2, axis=1)

    ct = _pack_dxf(centers.T.astype(_NP_DT))
    xt2 = _pack_dxf((2.0 * inputs).T.astype(_NP_DT))

    in_maps = []
    for i in range(N_CORES):
        sl = slice(i * NSH, (i + 1) * NSH)
        in_maps.append({"ct": ct, "xt": np.ascontiguousarray(xt2[:, :, sl])})

    nc = _get_nc()
    try:
        res = run_bass_kernel_spmd(
            nc, in_maps, core_ids=list(range(N_CORES)), trace=_trace
        )
    except ModuleNotFoundError:
        # NTFF trace glue is absent in some images; rerun without tracing
        res = run_bass_kernel_spmd(
            nc, in_maps, core_ids=list(range(N_CORES)), trace=False
        )
    if _trace:
        kernel.last_results = res

    # device returns the raw cross term [K, NSH] per core; add the exact
    # norm terms on the host
    cross = np.concatenate(
        [r["out"] for r in res.results], axis=1
    ).astype(np.float32)
    logits = cross - csq[:, None].astype(np.float32)
    logits -= xsq[None, :].astype(np.float32)
    return logits


# revision 12
# speedup vs baseline: 1.1714x; 1.1714x over previous
"""Trainium2 Bass kernel for nn_CentersDistance (retrieval_knn).

logits[k, n] = -||centers[k] - inputs[n]||^2
             = 2*(centers @ inputs.T)[k, n] - ||centers[k]||^2 - ||inputs[n]||^2

Strategy (8 NeuronCores, data-parallel over the N=8192 inputs):
  * device computes ONLY the cross term 2*c.x as 64 fp8e4m3 DoubleRow
    matmuls per core (contraction 256/MM via the [p, 2, f] interleave,
    2 fp8 MACs/cell/cycle -> 216 ns/MM at 2.4 GHz = the DR stream
    floor), PSUM fp32, copied to SBUF as fp16 and stored; the exact norm
    terms (float64 on host) are added on the host after gather.
  * group order in "quarters": (m0-3,h0), (m0-3,h1), (m4-7,h0),
    (m4-7,h1); within a quarter the matmuls run j-major (all 4 groups'
    DR-step j before step j+1) so the PE starts on the first 128 KB
    chunk pair as soon as it lands.
  * chunk-major data layout: both DRAM and SBUF hold each 128 KB chunk
    with 1 KB contiguous per partition, so every load descriptor is
    1 KB (512 B descriptors measurably cap the two HW-DGE rings at
    ~270 GB/s aggregate under 8-core contention).  ct chunks (j, m-half)
    on the Scalar ring, xt chunks (j, h) on the Sync ring, one
    semaphore per chunk.
  * PSUM->SBUF fp16 copies alternate DVE (even retirement order) / ACT
    (odd); stores alternate Sync (even) / ACT (odd).  PSUM bank-reuse
    waits are hoisted into the previous quarter's last matmul phase so
    they never stall the PE's LDWEIGHTS pull-ahead at a quarter boundary.
  * No final store wait: the NRT postamble's DMA drain covers the last
    in-flight stores.
  * N_WU throwaway matmuls on an uninitialized scratch tile keep the PE
    busy from preamble-end to first-chunk-arrival so the HAM clock gate
    (~3.4 us sustained-busy window) opens before the real matmuls start.
"""

import threading
from contextlib import ExitStack

import numpy as np
import ml_dtypes

import concourse.mybir as mybir
from concourse import bacc
from concourse.bass_utils import run_bass_kernel_spmd

N_CORES = 8
N, K, D = 8192, 1024, 1024
NSH = N // N_CORES  # per-core slab of inputs
P = 128             # SBUF partitions
NF = 512            # matmul moving free dim (one fp32 PSUM bank)

KS = D // P         # 8 contraction subtiles of 128
J = KS // 2         # 4 DoubleRow steps of 256
M_TILES = K // P    # 8 center tiles
H_TILES = NSH // NF # 2 moving-dim tiles

G = M_TILES * H_TILES  # 16 output groups of [128, 512]
N_WU = 8               # PE warm-up matmuls

# group order: quarters (m0-3,h0), (m0-3,h1), (m4-7,h0), (m4-7,h1)
GROUP_ORDER = (
    [(m, 0) for m in range(4)]
    + [(m, 1) for m in range(4)]
    + [(m, 0) for m in range(4, 8)]
    + [(m, 1) for m in range(4, 8)]
)

_DT = mybir.dt.float8e4
_NP_DT = ml_dtypes.float8_e4m3

_cache = threading.local()


# chunk-major layout: tensor [128, 16, 512]; chunk c occupies slots
# (2c, 2c+1); slot 2c+s holds contraction subtile (2j+s) for the chunk's
# 512-column slice.  ct chunk c = mhalf*4 + j; xt chunk c = h*4 + j.
def _ct_chunk(j, m):
    return (m // 4) * J + j


def _xt_chunk(j, h):
    return h * J + j


def _build_nc(dt=_DT, n_wu=N_WU):
    nc = bacc.Bacc(
        "TRN2", target_bir_lowering=False, debug=False, num_devices=N_CORES
    )
    ct = nc.dram_tensor("ct", [P, 2 * KS, NF], dt, kind="ExternalInput").ap()
    xt = nc.dram_tensor("xt", [P, 2 * KS, NF], dt, kind="ExternalInput").ap()
    out = nc.dram_tensor("out", [K, NSH], mybir.dt.float16, kind="ExternalOutput").ap()

    out_r = out.rearrange("(m p) n -> m p n", p=P)

    DR = mybir.MatmulPerfMode.DoubleRow

    with (
        nc.sbuf_tensor("wu_sb", [P, NF], dt) as wu_sb,
        nc.sbuf_tensor("ct_sb", [P, 2 * KS, NF], dt) as ct_sb,
        nc.sbuf_tensor("xt_sb", [P, 2 * KS, NF], dt) as xt_sb,
        nc.sbuf_tensor("ot_sb", [P, G * NF], mybir.dt.float16) as ot_sb,
        ExitStack() as stack,
        nc.semaphore("mm_sem") as mm_sem,
        nc.semaphore("dve_cp") as dve_cp,
        nc.semaphore("act_cp") as act_cp,
        nc.semaphore("dma_out") as dma_out,
        nc.Block() as block,
    ):
        ct_sems = [
            stack.enter_context(nc.semaphore(f"ct_sem{i}")) for i in range(2 * J)
        ]
        xt_sems = [
            stack.enter_context(nc.semaphore(f"xt_sem{i}")) for i in range(2 * J)
        ]
        ps = [
            stack.enter_context(nc.psum_tensor(f"ps{b}", [P, NF], mybir.dt.float32))
            for b in range(8)
        ]

        cp_sem = {0: dve_cp, 1: act_cp}  # order parity -> copy engine sem

        @block.sync
        def _(sync):
            for c in range(2 * J):  # xt chunks, consumption order
                sync.dma_start(
                    xt_sb[:, 2 * c : 2 * c + 2, :], xt[:, 2 * c : 2 * c + 2, :]
                ).then_inc(xt_sems[c], 16)
            # even-order stores, gated on the DVE copy
            for o, (m, h) in enumerate(GROUP_ORDER):
                if o % 2 != 0:
                    continue
                sync.wait_ge(dve_cp, (o // 2) + 1)
                sync.dma_start(
                    out_r[m][:, h * NF : (h + 1) * NF],
                    ot_sb[:, o * NF : (o + 1) * NF],
                ).then_inc(dma_out, 16)

        @block.scalar
        def _(scalar):
            for c in range(2 * J):  # ct chunks, consumption order
                scalar.dma_start(
                    ct_sb[:, 2 * c : 2 * c + 2, :], ct[:, 2 * c : 2 * c + 2, :]
                ).then_inc(ct_sems[c], 16)
            # odd-order copies (PSUM -> SBUF fp16) on ACT, each followed
            # by its own store (gated on the copy's semaphore: the DMA
            # descriptor fetch can race the ACT pipeline's SBUF write)
            for o, (m, h) in enumerate(GROUP_ORDER):
                if o % 2 != 1:
                    continue
                scalar.wait_ge(mm_sem, o + 1)
                nc.scalar.copy(
                    ot_sb[:, o * NF : (o + 1) * NF], ps[o % 8][:]
                ).then_inc(act_cp, 1)
                scalar.wait_ge(act_cp, (o // 2) + 1)
                scalar.dma_start(
                    out_r[m][:, h * NF : (h + 1) * NF],
                    ot_sb[:, o * NF : (o + 1) * NF],
                ).then_inc(dma_out, 16)

        @block.tensor
        def _(tensor):
            # warm-up: keep the PE busy from preamble-end to first-chunk
            # arrival so the HAM clock gate opens early.  wu_sb is
            # deliberately uninitialized; bank 7 is rewritten with
            # start=True by order-7's first matmul much later.
            for _ in range(n_wu):
                nc.tensor.matmul(
                    ps[7][:], wu_sb[:, 0:P], wu_sb[:], start=True, stop=True
                )
            ct_waited = set()
            xt_waited = set()
            for q in range(4):  # quarter
                quarter = list(enumerate(GROUP_ORDER))[4 * q : 4 * q + 4]
                for j in range(J):
                    for i, (o, (m, h)) in enumerate(quarter):
                        ci = _ct_chunk(j, m)
                        if ci not in ct_waited:
                            ct_waited.add(ci)
                            tensor.wait_ge(ct_sems[ci], 16)
                        xi = _xt_chunk(j, h)
                        if xi not in xt_waited:
                            xt_waited.add(xi)
                            tensor.wait_ge(xt_sems[xi], 16)
                        if j == J - 1 and q >= 1:
                            # hoisted PSUM bank-reuse wait for the NEXT
                            # quarter's order (o_next = o+4): its bank was
                            # last drained by order o_next-8's copy, done
                            # well before this point, so the wait is free
                            # here but would stall the LDWEIGHTS
                            # pull-ahead at the quarter boundary
                            o_next = 4 * (q + 1) + i
                            if o_next < G and o_next >= 8:
                                tensor.wait_ge(
                                    cp_sem[(o_next - 8) % 2],
                                    ((o_next - 8) // 2) + 1,
                                )
                        mm = nc.tensor.matmul(
                            ps[o % 8][:],
                            ct_sb[
                                :,
                                2 * _ct_chunk(j, m) : 2 * _ct_chunk(j, m) + 2,
                                (m % 4) * P : (m % 4 + 1) * P,
                            ],
                            xt_sb[:, 2 * _xt_chunk(j, h) : 2 * _xt_chunk(j, h) + 2, :],
                            start=(j == 0),
                            stop=(j == J - 1),
                            perf_mode=DR,
                        )
                        if j == J - 1:
                            mm.then_inc(mm_sem, 1)

        @block.vector
        def _(vector):
            # even-order copies (PSUM -> SBUF fp16) on the DVE
            for o, (m, h) in enumerate(GROUP_ORDER):
                if o % 2 != 0:
                    continue
                vector.wait_ge(mm_sem, o + 1)
                nc.vector.tensor_copy(
                    ot_sb[:, o * NF : (o + 1) * NF], ps[o % 8][:]
                ).then_inc(dve_cp, 1)

    nc.compile()
    return nc


def _get_nc():
    if not hasattr(_cache, "nc"):
        _cache.nc = _build_nc()
    return _cache.nc


def _pack_chunk_major(a_t):
    """[D, F] (F = 1024) -> [128, 16, 512] chunk-major fp8 layout.

    Slot 2c+s of the output holds contraction subtile 2j+s, column half
    fhalf, where c = fhalf*4 + j.  Contraction index d = ks*128 + p.
    """
    Dd, F = a_t.shape
    v = a_t.reshape(J, 2, P, 2, NF)        # [j, s, p, fhalf, f']
    v = v.transpose(2, 3, 0, 1, 4)         # [p, fhalf, j, s, f']
    return np.ascontiguousarray(v.reshape(P, 2 * KS, NF))


def kernel(inputs, centers, _trace=False):
    inputs = np.asarray(inputs, dtype=np.float32)
    centers = np.asarray(centers, dtype=np.float32)

    csq = np.sum(centers.astype(np.float64) ** 2, axis=1)
    xsq = np.sum(inputs.astype(np.float64) ** 2, axis=1)

    ct = _pack_chunk_major(centers.T.astype(_NP_DT))
    xt2 = (2.0 * inputs).T.astype(_NP_DT)

    in_maps = []
    for i in range(N_CORES):
        sl = slice(i * NSH, (i + 1) * NSH)
        in_maps.append(
            {"ct": ct, "xt": _pack_chunk_major(xt2[:, sl])}
        )

    nc = _get_nc()
    try:
        res = run_bass_kernel_spmd(
            nc, in_maps, core_ids=list(range(N_CORES)), trace=_trace
        )
    except ModuleNotFoundError:
        # NTFF trace glue is absent in some images; rerun without tracing
        res = run_bass_kernel_spmd(
            nc, in_maps, core_ids=list(range(N_CORES)), trace=False
        )
    if _trace:
        kernel.last_results = res

    # device returns the raw cross term [K, NSH] per core; add the exact
    # norm terms on the host
    cross = np.concatenate(
        [r["out"] for r in res.results], axis=1
    ).astype(np.float32)
    logits = cross - csq[:, None].astype(np.float32)
    logits -= xsq[None, :].astype(np.float32)
    return logits
